# revision 15
# baseline (speedup 1.0000x reference)
"""CondConv (per-sample routed 3x3 conv) on 8 Trainium2 NeuronCores.

Reference computation (all fp32):
    gap     = mean(x, axis=(2,3))                    [B, CIN]
    routing = sigmoid(gap @ W_att.T + b_att)         [B, E]
    ker     = einsum('be,eoihw->boihw', routing, convs)
    out[b]  = conv2d(x[b], ker[b], stride 1, pad 1)  [B, COUT, 56, 56]

Sharding (B=32, COUT=256 across 8 cores): 4 core-pairs; pair p owns
samples 8p..8p+7 (batch data-parallel), and within a pair each core
computes one half of COUT (128 channels). Halving COUT per core halves
the resident expert bank so the whole pipeline stays fp32 in SBUF.

Per-core program (SPMD — same program, different data):
  - expert bank convsT [8e][2c][128cin, 9*128] resident in SBUF
  - per sample: DMA padded x -> GAP via ScalarE accum -> routing
    (2 small matmuls + sigmoid + diag/broadcast matmul) -> VectorE mixes
    the per-sample kernel with fused scalar_tensor_tensor -> conv as
    2c*9shift*7tile accumulating fp32r matmuls (N=448) -> ScalarE drains
    PSUM -> DMA out.
"""

import numpy as np

B, CIN, H, W = 32, 256, 56, 56
COUT, KK, E = 256, 3, 8
HP, WP = H + 2, W + 2          # zero-padded input plane
PHW = HP * WP                  # 3364
NSH = KK * KK                  # 9 shifts
CHUNKS = 2                     # CIN = 2 * 128
MHALF = COUT // 2              # couts per core
ROWS_PER_TILE = 8              # output rows per matmul tile
NTILES = H // ROWS_PER_TILE    # 7
NFREE = ROWS_PER_TILE * W      # 448
NCORES = 8
SAMPLES_PER_CORE = B // (NCORES // 2)  # 8

_cached = {}


def _build_program():
    import concourse.bacc as bacc
    import concourse.mybir as mybir
    from concourse.tile import TileContext

    f32 = mybir.dt.float32
    f32r = mybir.dt.float32r
    Alu = mybir.AluOpType
    Act = mybir.ActivationFunctionType

    nc = bacc.Bacc(None, target_bir_lowering=False)

    xpad_d = nc.declare_dram_parameter(
        "xpad", [SAMPLES_PER_CORE, CHUNKS, 128, PHW], f32, isOutput=False)
    convsT_d = nc.declare_dram_parameter(
        "convsT", [E, CHUNKS, 128, NSH * 128], f32, isOutput=False)
    watt_d = nc.declare_dram_parameter("watt", [CHUNKS, 128, E], f32, isOutput=False)
    batt_d = nc.declare_dram_parameter("batt", [E, 1], f32, isOutput=False)
    ones8_d = nc.declare_dram_parameter("ones8", [E, 128], f32, isOutput=False)
    ident8_d = nc.declare_dram_parameter("ident8", [E, E], f32, isOutput=False)
    out_d = nc.declare_dram_parameter(
        "out", [SAMPLES_PER_CORE, MHALF, H, W], f32, isOutput=True)

    with TileContext(nc) as tc:
        with (
            tc.tile_pool(name="resident", bufs=1) as res_pool,
            tc.tile_pool(name="xp", bufs=3) as xp_pool,
            tc.tile_pool(name="kt", bufs=3) as kt_pool,
            tc.tile_pool(name="small", bufs=3) as small_pool,
            tc.tile_pool(name="outsb", bufs=4) as out_pool,
            tc.tile_pool(name="cpsum", bufs=1, space="PSUM") as cps_pool,
            tc.tile_pool(name="rpsum", bufs=1, space="PSUM") as rps_pool,
        ):
            # ---- resident tiles -------------------------------------------------
            watt_sb = []
            for c in range(CHUNKS):
                t = res_pool.tile([128, E], f32, name=f"watt{c}", tag=f"watt{c}")
                nc.sync.dma_start(out=t[:], in_=watt_d[c])
                watt_sb.append(t)
            batt_sb = res_pool.tile([E, 1], f32, name="batt", tag="batt")
            nc.sync.dma_start(out=batt_sb[:], in_=batt_d[:])
            ones8_sb = res_pool.tile([E, 128], f32, name="ones8", tag="ones8")
            nc.sync.dma_start(out=ones8_sb[:], in_=ones8_d[:])
            ident8_sb = res_pool.tile([E, E], f32, name="ident8", tag="ident8")
            nc.sync.dma_start(out=ident8_sb[:], in_=ident8_d[:])
            # broadcast routing weights: scal[:, 8*b+e] = r_be on every partition
            scal_sb = res_pool.tile([128, SAMPLES_PER_CORE * E], f32,
                                    name="scal", tag="scal")

            def emit_load(b):
                """DMA padded input + GAP/rounding pass for sample b.

                ScalarE in-place Copy rounds fp32 -> fp32r for the conv
                matmuls (walrus requires fp32r matmul inputs to come from a
                rounding producer) and its accum_out yields the GAP row sums
                in the same pass.
                """
                xp = []
                gs = []
                for c in range(CHUNKS):
                    t = xp_pool.tile([128, PHW], f32r, name=f"xp{c}", tag=f"xp{c}")
                    nc.gpsimd.dma_start(out=t[:], in_=xpad_d[b, c])
                    g = small_pool.tile([128, 1], f32, name=f"gs{c}", tag=f"gs{c}")
                    nc.scalar.activation(out=t[:], in_=t[:], func=Act.Copy,
                                         accum_out=g[:])
                    xp.append(t)
                    gs.append(g)
                return xp, gs

            def emit_route_mix(b, xp, gs):
                """Routing + per-sample kernel mix for sample b."""
                # routing: logits[e] = sum_cin gap*W/3136 + b (1/3136 in watt)
                ps_r = rps_pool.tile([E, 1], f32, name="ps_r", tag="rps")
                for c in range(CHUNKS):
                    nc.tensor.matmul(ps_r[:], watt_sb[c][:], gs[c][:],
                                     start=(c == 0), stop=(c == CHUNKS - 1))
                rout = small_pool.tile([E, 1], f32, name="rout", tag="rout")
                nc.scalar.activation(out=rout[:], in_=ps_r[:], func=Act.Sigmoid,
                                     bias=batt_sb[:, 0:1], scale=1.0)
                # diag(r) then ones.T @ diag(r) broadcasts r to all partitions
                diag = small_pool.tile([E, E], f32, name="diag", tag="diag")
                nc.vector.tensor_scalar_mul(out=diag[:], in0=ident8_sb[:],
                                            scalar1=rout[:, 0:1])
                ps_b = rps_pool.tile([128, E], f32, name="ps_b", tag="rps")
                nc.tensor.matmul(ps_b[:], ones8_sb[:], diag[:], start=True, stop=True)
                nc.scalar.activation(out=scal_sb[:, b * E:(b + 1) * E], in_=ps_b[:],
                                     func=Act.Copy)

                # mix on VectorE:
                # kerT[c][cin, s*128+m] = sum_e r_be * convsT[e][c][cin, s*128+m]
                kt = []
                for c in range(CHUNKS):
                    k = kt_pool.tile([128, NSH * 128], f32r, name=f"kt{c}", tag=f"kt{c}")
                    nc.vector.tensor_scalar_mul(
                        out=k[:], in0=convsT_sb[0][c][:],
                        scalar1=scal_sb[:, b * E:b * E + 1])
                    for e in range(1, E):
                        nc.vector.scalar_tensor_tensor(
                            out=k[:], in0=convsT_sb[e][c][:],
                            scalar=scal_sb[:, b * E + e:b * E + e + 1],
                            in1=k[:], op0=Alu.mult, op1=Alu.add)
                    kt.append(k)
                return kt

            def emit_conv(b, xp, kt):
                """Conv for sample b: accumulate 2c*9shift into 7 PSUM tiles."""
                cps = [cps_pool.tile([128, NFREE], f32, name=f"cps{n}", tag=f"cps{n}")
                       for n in range(NTILES)]
                for c in range(CHUNKS):
                    x3 = xp[c].rearrange("p (r q) -> p r q", q=WP)
                    for s in range(NSH):
                        dh, dw = s // KK, s % KK
                        lhsT = kt[c][:, s * 128:(s + 1) * 128]
                        first = (c == 0 and s == 0)
                        last = (c == CHUNKS - 1 and s == NSH - 1)
                        for n in range(NTILES):
                            rhs = x3[:, n * ROWS_PER_TILE + dh:
                                     n * ROWS_PER_TILE + dh + ROWS_PER_TILE,
                                     dw:dw + W]
                            nc.tensor.matmul(cps[n][:], lhsT, rhs,
                                             start=first, stop=last)
                # drain PSUM on VectorE (its mix work ends before conv does;
                # ScalarE's gap passes would block drains behind input DMAs)
                for n in range(NTILES):
                    o = out_pool.tile([128, NFREE], f32, name="osb", tag="osb")
                    nc.vector.tensor_copy(out=o[:], in_=cps[n][:])
                    nc.sync.dma_start(
                        out=out_d[b, :, n * ROWS_PER_TILE:(n + 1) * ROWS_PER_TILE, :],
                        in_=o[:])

            # Software-pipelined emission: sample b's load/routing/mix is
            # issued LOOKAHEAD samples before conv(b), so the tiny routing
            # matmuls of upcoming samples sit ahead of the long conv bursts
            # in the PE queue and the VectorE mix overlaps earlier convs
            # instead of serializing after them. The prologue orders DMAs so
            # xp(0) and the conv bank arrive before xp(1..).
            LOOKAHEAD = 3
            ld0 = emit_load(0)
            convsT_sb = [[None] * CHUNKS for _ in range(E)]
            for c in range(CHUNKS):
                for e in range(E):
                    t = res_pool.tile([128, NSH * 128], f32,
                                      name=f"cv_{e}_{c}", tag=f"cv_{e}_{c}")
                    nc.sync.dma_start(out=t[:], in_=convsT_d[e, c])
                    convsT_sb[e][c] = t
            stash = {0: (ld0[0], emit_route_mix(0, *ld0))}
            for b in range(1, min(LOOKAHEAD, SAMPLES_PER_CORE)):
                ld = emit_load(b)
                stash[b] = (ld[0], emit_route_mix(b, *ld))
            for b in range(SAMPLES_PER_CORE):
                if b + LOOKAHEAD < SAMPLES_PER_CORE:
                    ld = emit_load(b + LOOKAHEAD)
                    stash[b + LOOKAHEAD] = (ld[0], emit_route_mix(b + LOOKAHEAD, *ld))
                emit_conv(b, *stash.pop(b))

    nc.compile()
    return nc


def _prep_core_inputs(x, convs, W_att, b_att):
    """Host-side shard/layout prep. Returns list of 8 per-core input dicts."""
    f32 = np.float32
    # padded input, cin split into 2 chunks of 128
    xpad = np.zeros((B, CHUNKS, 128, HP, WP), dtype=f32)
    xpad[:, :, :, 1:H + 1, 1:W + 1] = np.ascontiguousarray(x, dtype=f32).reshape(
        B, CHUNKS, 128, H, W)
    xpad = xpad.reshape(B, CHUNKS, 128, PHW)

    # convsT[half][e, c, cin, s*128 + m] = convs[e, half*128+m, c*128+cin, kh, kw]
    cv = np.ascontiguousarray(convs, dtype=f32).reshape(E, 2, MHALF, CHUNKS, 128, NSH)
    convsT_halves = [
        np.ascontiguousarray(cv[:, h].transpose(0, 2, 3, 4, 1).reshape(
            E, CHUNKS, 128, NSH * 128))
        for h in range(2)
    ]

    watt = np.ascontiguousarray(
        (np.asarray(W_att, dtype=f32).T / f32(H * W)).reshape(CHUNKS, 128, E))
    batt = np.ascontiguousarray(np.asarray(b_att, dtype=f32).reshape(E, 1))
    ones8 = np.ones((E, 128), dtype=f32)
    ident8 = np.eye(E, dtype=f32)

    in_maps = []
    for k in range(NCORES):
        pair, half = k // 2, k % 2
        sl = slice(pair * SAMPLES_PER_CORE, (pair + 1) * SAMPLES_PER_CORE)
        in_maps.append({
            "xpad": np.ascontiguousarray(xpad[sl]),
            "convsT": convsT_halves[half],
            "watt": watt,
            "batt": batt,
            "ones8": ones8,
            "ident8": ident8,
        })
    return in_maps


def _assemble_output(results):
    out = np.empty((B, COUT, H, W), dtype=np.float32)
    for k in range(NCORES):
        pair, half = k // 2, k % 2
        sl = slice(pair * SAMPLES_PER_CORE, (pair + 1) * SAMPLES_PER_CORE)
        out[sl, half * MHALF:(half + 1) * MHALF] = results[k]["out"]
    return out


def kernel(x, convs, W_att, b_att):
    from concourse.bass_utils import run_bass_kernel_spmd

    if "nc" not in _cached:
        _cached["nc"] = _build_program()
    in_maps = _prep_core_inputs(x, convs, W_att, b_att)
    res = run_bass_kernel_spmd(_cached["nc"], in_maps, core_ids=list(range(NCORES)))
    return _assemble_output(res.results)


# revision 16
# speedup vs baseline: 1.0742x; 1.0742x over previous
"""CondConv (per-sample routed 3x3 conv) on 8 Trainium2 NeuronCores.

Reference computation (all fp32):
    gap     = mean(x, axis=(2,3))                    [B, CIN]
    routing = sigmoid(gap @ W_att.T + b_att)         [B, E]
    ker     = einsum('be,eoihw->boihw', routing, convs)
    out[b]  = conv2d(x[b], ker[b], stride 1, pad 1)  [B, COUT, 56, 56]

Sharding (B=32, COUT=256 across 8 cores): 4 core-pairs; pair p owns
samples 8p..8p+7 (batch data-parallel), and within a pair each core
computes one half of COUT (128 channels). Halving COUT per core halves
the resident expert bank so the whole pipeline stays fp32 in SBUF.

Per-core program (SPMD — same program, different data):
  - expert bank convsT [8e][2c][128cin, 9*128] resident in SBUF
  - per sample: DMA padded x -> GAP + fp32r rounding via one ScalarE
    pass -> routing on DVE/GPSIMD/ScalarE (no TensorE, so the PE queue
    is pure conv) -> VectorE mixes the per-sample kernel with fused
    scalar_tensor_tensor -> conv as 2c*9shift*7tile accumulating fp32r
    matmuls (N=448, full PE rate) -> VectorE drains PSUM -> DMA out.
  - emission is software-pipelined so mix(b+2) overlaps conv(b..b+1)
    and input DMAs run 3 samples ahead.
"""

import numpy as np

B, CIN, H, W = 32, 256, 56, 56
COUT, KK, E = 256, 3, 8
HP, WP = H + 2, W + 2          # zero-padded input plane
PHW = HP * WP                  # 3364
NSH = KK * KK                  # 9 shifts
CHUNKS = 2                     # CIN = 2 * 128
MHALF = COUT // 2              # couts per core
ROWS_PER_TILE = 8              # output rows per matmul tile
NTILES = H // ROWS_PER_TILE    # 7
NFREE = ROWS_PER_TILE * W      # 448
NCORES = 8
SAMPLES_PER_CORE = B // (NCORES // 2)  # 8

_cached = {}


def _build_program():
    import concourse.bacc as bacc
    import concourse.bass_isa as bass_isa
    import concourse.mybir as mybir
    from concourse.tile import TileContext

    f32 = mybir.dt.float32
    f32r = mybir.dt.float32r
    Alu = mybir.AluOpType
    Act = mybir.ActivationFunctionType

    nc = bacc.Bacc(None, target_bir_lowering=False)

    xpad_d = nc.declare_dram_parameter(
        "xpad", [SAMPLES_PER_CORE, CHUNKS, 128, PHW], f32, isOutput=False)
    convsT_d = nc.declare_dram_parameter(
        "convsT", [E, CHUNKS, 128, NSH * 128], f32, isOutput=False)
    watt_d = nc.declare_dram_parameter("watt", [CHUNKS, 128, E], f32, isOutput=False)
    battb_d = nc.declare_dram_parameter("battb", [128, E], f32, isOutput=False)
    out_d = nc.declare_dram_parameter(
        "out", [SAMPLES_PER_CORE, MHALF, H, W], f32, isOutput=True)

    with TileContext(nc) as tc:
        with (
            tc.tile_pool(name="resident", bufs=1) as res_pool,
            tc.tile_pool(name="xp", bufs=3) as xp_pool,
            tc.tile_pool(name="kt", bufs=3) as kt_pool,
            tc.tile_pool(name="small", bufs=3) as small_pool,
            tc.tile_pool(name="outsb", bufs=4) as out_pool,
            tc.tile_pool(name="cpsum", bufs=1, space="PSUM") as cps_pool,
        ):
            # ---- small resident tiles -------------------------------------
            watt_sb = []
            for c in range(CHUNKS):
                t = res_pool.tile([128, E], f32, name=f"watt{c}", tag=f"watt{c}")
                nc.sync.dma_start(out=t[:], in_=watt_d[c])
                watt_sb.append(t)
            battb_sb = res_pool.tile([128, E], f32, name="battb", tag="battb")
            nc.sync.dma_start(out=battb_sb[:], in_=battb_d[:])
            # broadcast routing weights: scal[:, 8*b+e] = r_be on every partition
            scal_sb = res_pool.tile([128, SAMPLES_PER_CORE * E], f32,
                                    name="scal", tag="scal")

            convsT_sb = [[None] * CHUNKS for _ in range(E)]

            def emit_load(b):
                """DMA padded input + GAP/rounding pass for sample b.

                ScalarE in-place Copy rounds fp32 -> fp32r for the conv
                matmuls (walrus requires fp32r matmul inputs to come from a
                rounding producer) and its accum_out yields the GAP row sums
                in the same pass.
                """
                xp = []
                gs = []
                for c in range(CHUNKS):
                    t = xp_pool.tile([128, PHW], f32r, name=f"xp{c}", tag=f"xp{c}")
                    nc.gpsimd.dma_start(out=t[:], in_=xpad_d[b, c])
                    g = small_pool.tile([128, 1], f32, name=f"gs{c}", tag=f"gs{c}")
                    nc.scalar.activation(out=t[:], in_=t[:], func=Act.Copy,
                                         accum_out=g[:])
                    xp.append(t)
                    gs.append(g)
                return xp, gs

            def emit_route_mix(b, gs):
                """Routing (DVE/GPSIMD/ScalarE only) + kernel mix for sample b.

                logits[e] = sum_cin gap[cin] * W_att[e,cin] / 3136 + b_att[e]
                (the 1/3136 is folded into watt host-side). Per-partition
                products on DVE, cross-partition sum on GPSIMD, sigmoid on
                ScalarE -- the TensorE queue stays pure conv.
                """
                t0 = small_pool.tile([128, E], f32, name="t0", tag="t0")
                nc.vector.tensor_scalar_mul(out=t0[:], in0=watt_sb[0][:],
                                            scalar1=gs[0][:, 0:1])
                t1 = small_pool.tile([128, E], f32, name="t1", tag="t1")
                nc.vector.scalar_tensor_tensor(
                    out=t1[:], in0=watt_sb[1][:], scalar=gs[1][:, 0:1],
                    in1=t0[:], op0=Alu.mult, op1=Alu.add)
                red = small_pool.tile([128, E], f32, name="red", tag="red")
                nc.gpsimd.partition_all_reduce(red[:], t1[:], channels=128,
                                               reduce_op=bass_isa.ReduceOp.add)
                red2 = small_pool.tile([128, E], f32, name="red2", tag="red2")
                nc.vector.tensor_add(out=red2[:], in0=red[:], in1=battb_sb[:])
                nc.scalar.activation(out=scal_sb[:, b * E:(b + 1) * E],
                                     in_=red2[:], func=Act.Sigmoid)

                # mix on VectorE:
                # kerT[c][cin, s*128+m] = sum_e r_be * convsT[e][c][cin, s*128+m]
                kt = []
                for c in range(CHUNKS):
                    k = kt_pool.tile([128, NSH * 128], f32r, name=f"kt{c}", tag=f"kt{c}")
                    nc.vector.tensor_scalar_mul(
                        out=k[:], in0=convsT_sb[0][c][:],
                        scalar1=scal_sb[:, b * E:b * E + 1])
                    for e in range(1, E):
                        nc.vector.scalar_tensor_tensor(
                            out=k[:], in0=convsT_sb[e][c][:],
                            scalar=scal_sb[:, b * E + e:b * E + e + 1],
                            in1=k[:], op0=Alu.mult, op1=Alu.add)
                    kt.append(k)
                return kt

            def emit_conv(b, xp, kt):
                """Conv for sample b: accumulate 2c*9shift into 7 PSUM tiles,
                then drain on VectorE and store."""
                cps = [cps_pool.tile([128, NFREE], f32, name=f"cps{n}",
                                     tag=f"cps{n}", bufs=2 if n == 0 else 1)
                       for n in range(NTILES)]
                for c in range(CHUNKS):
                    x3 = xp[c].rearrange("p (r q) -> p r q", q=WP)
                    for s in range(NSH):
                        dh, dw = s // KK, s % KK
                        lhsT = kt[c][:, s * 128:(s + 1) * 128]
                        first = (c == 0 and s == 0)
                        last = (c == CHUNKS - 1 and s == NSH - 1)
                        for n in range(NTILES):
                            rhs = x3[:, n * ROWS_PER_TILE + dh:
                                     n * ROWS_PER_TILE + dh + ROWS_PER_TILE,
                                     dw:dw + W]
                            nc.tensor.matmul(cps[n][:], lhsT, rhs,
                                             start=first, stop=last)
                for n in range(NTILES):
                    o = out_pool.tile([128, NFREE], f32, name="osb", tag="osb")
                    nc.vector.tensor_copy(out=o[:], in_=cps[n][:])
                    nc.sync.dma_start(
                        out=out_d[b, :, n * ROWS_PER_TILE:(n + 1) * ROWS_PER_TILE, :],
                        in_=o[:])

            # ---- software-pipelined emission ------------------------------
            # loads run 3 samples ahead of conv, routing+mix 2 ahead; the
            # conv-bank DMAs are ordered chunk0-first so mix(0) can start as
            # soon as chunk0 and xp(0) land.
            S = SAMPLES_PER_CORE
            loads = {0: emit_load(0)}
            for c in range(CHUNKS):
                for e in range(E):
                    t = res_pool.tile([128, NSH * 128], f32,
                                      name=f"cv_{e}_{c}", tag=f"cv_{e}_{c}")
                    nc.sync.dma_start(out=t[:], in_=convsT_d[e, c])
                    convsT_sb[e][c] = t
            kts = {0: emit_route_mix(0, loads[0][1])}
            loads[1] = emit_load(1)
            kts[1] = emit_route_mix(1, loads[1][1])
            loads[2] = emit_load(2)
            for b in range(S):
                emit_conv(b, loads.pop(b)[0], kts.pop(b))
                if b + 2 < S:
                    kts[b + 2] = emit_route_mix(b + 2, loads[b + 2][1])
                if b + 3 < S:
                    loads[b + 3] = emit_load(b + 3)

    nc.compile()
    return nc


def _prep_core_inputs(x, convs, W_att, b_att):
    """Host-side shard/layout prep. Returns list of 8 per-core input dicts."""
    f32 = np.float32
    # padded input, cin split into 2 chunks of 128
    xpad = np.zeros((B, CHUNKS, 128, HP, WP), dtype=f32)
    xpad[:, :, :, 1:H + 1, 1:W + 1] = np.ascontiguousarray(x, dtype=f32).reshape(
        B, CHUNKS, 128, H, W)
    xpad = xpad.reshape(B, CHUNKS, 128, PHW)

    # convsT[half][e, c, cin, s*128 + m] = convs[e, half*128+m, c*128+cin, kh, kw]
    cv = np.ascontiguousarray(convs, dtype=f32).reshape(E, 2, MHALF, CHUNKS, 128, NSH)
    convsT_halves = [
        np.ascontiguousarray(cv[:, h].transpose(0, 2, 3, 4, 1).reshape(
            E, CHUNKS, 128, NSH * 128))
        for h in range(2)
    ]

    watt = np.ascontiguousarray(
        (np.asarray(W_att, dtype=f32).T / f32(H * W)).reshape(CHUNKS, 128, E))
    battb = np.ascontiguousarray(
        np.broadcast_to(np.asarray(b_att, dtype=f32), (128, E)))

    in_maps = []
    for k in range(NCORES):
        pair, half = k // 2, k % 2
        sl = slice(pair * SAMPLES_PER_CORE, (pair + 1) * SAMPLES_PER_CORE)
        in_maps.append({
            "xpad": np.ascontiguousarray(xpad[sl]),
            "convsT": convsT_halves[half],
            "watt": watt,
            "battb": battb,
        })
    return in_maps


def _assemble_output(results):
    out = np.empty((B, COUT, H, W), dtype=np.float32)
    for k in range(NCORES):
        pair, half = k // 2, k % 2
        sl = slice(pair * SAMPLES_PER_CORE, (pair + 1) * SAMPLES_PER_CORE)
        out[sl, half * MHALF:(half + 1) * MHALF] = results[k]["out"]
    return out


def kernel(x, convs, W_att, b_att):
    from concourse.bass_utils import run_bass_kernel_spmd

    if "nc" not in _cached:
        _cached["nc"] = _build_program()
    in_maps = _prep_core_inputs(x, convs, W_att, b_att)
    res = run_bass_kernel_spmd(_cached["nc"], in_maps, core_ids=list(range(NCORES)))
    return _assemble_output(res.results)


# revision 20
# speedup vs baseline: 1.2926x; 1.2033x over previous
"""CondConv (per-sample routed 3x3 conv) on 8 Trainium2 NeuronCores.

Reference computation (all fp32):
    gap     = mean(x, axis=(2,3))                    [B, CIN]
    routing = sigmoid(gap @ W_att.T + b_att)         [B, E]
    ker     = einsum('be,eoihw->boihw', routing, convs)
    out[b]  = conv2d(x[b], ker[b], stride 1, pad 1)  [B, COUT, 56, 56]

Sharding (B=32, COUT=256 across 8 cores): 4 core-pairs; pair p owns
samples 8p..8p+7 (batch data-parallel), and within a pair each core
computes one half of COUT (128 channels). Halving COUT per core halves
the resident expert bank so the whole pipeline stays fp32 in SBUF.

Per-core program (SPMD — same program, different data):
  - expert bank convsT [8e][2c][128cin, 9*128] resident in SBUF
  - per sample: DMA padded x -> GAP + fp32r rounding via one ScalarE
    pass -> routing on DVE/GPSIMD/ScalarE (no TensorE, so the PE queue
    is pure conv) -> VectorE mixes the per-sample kernel with fused
    scalar_tensor_tensor -> conv as 2c*9shift*7tile accumulating fp32r
    matmuls (N=448, full PE rate) -> VectorE drains PSUM -> DMA out.
  - emission is software-pipelined so mix(b+2) overlaps conv(b..b+1)
    and input DMAs run 3 samples ahead.
"""

import numpy as np

B, CIN, H, W = 32, 256, 56, 56
COUT, KK, E = 256, 3, 8
HP, WP = H + 2, W + 2          # zero-padded input plane
PHW = HP * WP                  # 3364
NSH = KK * KK                  # 9 shifts
CHUNKS = 2                     # CIN = 2 * 128
MHALF = COUT // 2              # couts per core
ROWS_PER_TILE = 8              # output rows per matmul tile
NTILES = H // ROWS_PER_TILE    # 7
NFREE = ROWS_PER_TILE * W      # 448
NCORES = 8
SAMPLES_PER_CORE = B // (NCORES // 2)  # 8

_cached = {}


def _build_program():
    import concourse.bacc as bacc
    import concourse.bass_isa as bass_isa
    import concourse.mybir as mybir
    from concourse.tile import TileContext

    f32 = mybir.dt.float32
    f32r = mybir.dt.float32r
    Alu = mybir.AluOpType
    Act = mybir.ActivationFunctionType

    nc = bacc.Bacc(None, target_bir_lowering=False)

    xpad_d = nc.declare_dram_parameter(
        "xpad", [SAMPLES_PER_CORE, CHUNKS, 128, PHW], f32r, isOutput=False)
    convsT_d = nc.declare_dram_parameter(
        "convsT", [E, CHUNKS, 128, NSH * 128], f32, isOutput=False)
    watt_d = nc.declare_dram_parameter("watt", [CHUNKS, 128, E], f32, isOutput=False)
    battb_d = nc.declare_dram_parameter("battb", [128, E], f32, isOutput=False)
    out_d = nc.declare_dram_parameter(
        "out", [SAMPLES_PER_CORE, MHALF, H, W], f32, isOutput=True)

    with TileContext(nc) as tc:
        with (
            tc.tile_pool(name="resident", bufs=1) as res_pool,
            tc.tile_pool(name="xp", bufs=3) as xp_pool,
            tc.tile_pool(name="kt", bufs=3) as kt_pool,
            tc.tile_pool(name="small", bufs=3) as small_pool,
            tc.tile_pool(name="outsb", bufs=4) as out_pool,
            tc.tile_pool(name="cpsum", bufs=1, space="PSUM") as cps_pool,
        ):
            # ---- small resident tiles -------------------------------------
            watt_sb = []
            for c in range(CHUNKS):
                t = res_pool.tile([128, E], f32, name=f"watt{c}", tag=f"watt{c}")
                nc.sync.dma_start(out=t[:], in_=watt_d[c])
                watt_sb.append(t)
            battb_sb = res_pool.tile([128, E], f32, name="battb", tag="battb")
            nc.sync.dma_start(out=battb_sb[:], in_=battb_d[:])
            # broadcast routing weights: scal[:, 8*b+e] = r_be on every partition
            scal_sb = res_pool.tile([128, SAMPLES_PER_CORE * E], f32,
                                    name="scal", tag="scal")

            convsT_sb = [[None] * CHUNKS for _ in range(E)]

            def emit_load(b):
                """DMA padded input + GAP/rounding pass for sample b.

                ScalarE in-place Copy rounds fp32 -> fp32r for the conv
                matmuls (walrus requires fp32r matmul inputs to come from a
                rounding producer) and its accum_out yields the GAP row sums
                in the same pass.
                """
                xp = []
                gs = []
                quarter = PHW // 4  # 841
                for c in range(CHUNKS):
                    t = xp_pool.tile([128, PHW], f32r, name=f"xp{c}", tag=f"xp{c}")
                    # split across 4 DMA queues so the load completes fast
                    # once the buffer slot frees up
                    for j in range(4):
                        nc.sync.dma_start(
                            out=t[:, j * quarter:(j + 1) * quarter],
                            in_=xpad_d[b, c, :, j * quarter:(j + 1) * quarter])
                    g = small_pool.tile([128, 1], f32, name=f"gs{c}", tag=f"gs{c}")
                    nc.scalar.activation(out=t[:], in_=t[:], func=Act.Copy,
                                         accum_out=g[:])
                    xp.append(t)
                    gs.append(g)
                return xp, gs

            def emit_route_mix(b, gs):
                """Routing (DVE/GPSIMD/ScalarE only) + kernel mix for sample b.

                logits[e] = sum_cin gap[cin] * W_att[e,cin] / 3136 + b_att[e]
                (the 1/3136 is folded into watt host-side). Per-partition
                products on DVE, cross-partition sum on GPSIMD, sigmoid on
                ScalarE -- the TensorE queue stays pure conv.
                """
                t0 = small_pool.tile([128, E], f32, name="t0", tag="t0")
                nc.vector.tensor_scalar_mul(out=t0[:], in0=watt_sb[0][:],
                                            scalar1=gs[0][:, 0:1])
                t1 = small_pool.tile([128, E], f32, name="t1", tag="t1")
                nc.vector.scalar_tensor_tensor(
                    out=t1[:], in0=watt_sb[1][:], scalar=gs[1][:, 0:1],
                    in1=t0[:], op0=Alu.mult, op1=Alu.add)
                red = small_pool.tile([128, E], f32, name="red", tag="red")
                nc.gpsimd.partition_all_reduce(red[:], t1[:], channels=128,
                                               reduce_op=bass_isa.ReduceOp.add)
                red2 = small_pool.tile([128, E], f32, name="red2", tag="red2")
                nc.vector.tensor_add(out=red2[:], in0=red[:], in1=battb_sb[:])
                nc.scalar.activation(out=scal_sb[:, b * E:(b + 1) * E],
                                     in_=red2[:], func=Act.Sigmoid)

                # mix on VectorE:
                # kerT[c][cin, s*128+m] = sum_e r_be * convsT[e][c][cin, s*128+m]
                kt = []
                for c in range(CHUNKS):
                    k = kt_pool.tile([128, NSH * 128], f32r, name=f"kt{c}", tag=f"kt{c}")
                    nc.vector.tensor_scalar_mul(
                        out=k[:], in0=convsT_sb[0][c][:],
                        scalar1=scal_sb[:, b * E:b * E + 1])
                    for e in range(1, E):
                        nc.vector.scalar_tensor_tensor(
                            out=k[:], in0=convsT_sb[e][c][:],
                            scalar=scal_sb[:, b * E + e:b * E + e + 1],
                            in1=k[:], op0=Alu.mult, op1=Alu.add)
                    kt.append(k)
                return kt

            def emit_conv(b, xp, kt):
                """Conv for sample b: accumulate 2c*9shift into 7 PSUM tiles,
                then drain on VectorE and store."""
                cps = [cps_pool.tile([128, NFREE], f32, name=f"cps{n}",
                                     tag=f"cps{n}", bufs=2 if n == 0 else 1)
                       for n in range(NTILES)]
                for c in range(CHUNKS):
                    x3 = xp[c].rearrange("p (r q) -> p r q", q=WP)
                    for s in range(NSH):
                        dh, dw = s // KK, s % KK
                        lhsT = kt[c][:, s * 128:(s + 1) * 128]
                        first = (c == 0 and s == 0)
                        last = (c == CHUNKS - 1 and s == NSH - 1)
                        for n in range(NTILES):
                            rhs = x3[:, n * ROWS_PER_TILE + dh:
                                     n * ROWS_PER_TILE + dh + ROWS_PER_TILE,
                                     dw:dw + W]
                            nc.tensor.matmul(cps[n][:], lhsT, rhs,
                                             start=first, stop=last)
                # drains alternate between VectorE and ScalarE so the 7 PSUM
                # banks free up at ~2x one engine's copy rate right as the
                # next sample's matmuls want them
                for n in range(NTILES):
                    o = out_pool.tile([128, NFREE], f32, name="osb", tag="osb")
                    eng = nc.vector if n % 2 == 0 else nc.scalar
                    if eng is nc.vector:
                        eng.tensor_copy(out=o[:], in_=cps[n][:])
                    else:
                        eng.activation(out=o[:], in_=cps[n][:], func=Act.Copy)
                    nc.sync.dma_start(
                        out=out_d[b, :, n * ROWS_PER_TILE:(n + 1) * ROWS_PER_TILE, :],
                        in_=o[:])

            # ---- software-pipelined emission ------------------------------
            # All loads ride the same HWDGE rings as the conv bank, so queue
            # FIFO order enforces: xp(0), bank chunk0, bank chunk1, xp(1),
            # xp(2), then steady-state loads 3 samples ahead. routing+mix(b+2)
            # is emitted after conv(b) so the mix overlaps conv(b) on VectorE,
            # with drains at the queue head for prompt PSUM recycling.
            S = SAMPLES_PER_CORE
            loads = {0: emit_load(0)}
            for c in range(CHUNKS):
                for e in range(E):
                    t = res_pool.tile([128, NSH * 128], f32,
                                      name=f"cv_{e}_{c}", tag=f"cv_{e}_{c}")
                    nc.sync.dma_start(out=t[:], in_=convsT_d[e, c])
                    convsT_sb[e][c] = t
            kts = {0: emit_route_mix(0, loads[0][1])}
            loads[1] = emit_load(1)
            kts[1] = emit_route_mix(1, loads[1][1])
            loads[2] = emit_load(2)
            for b in range(S):
                emit_conv(b, loads.pop(b)[0], kts.pop(b))
                if b + 2 < S:
                    kts[b + 2] = emit_route_mix(b + 2, loads[b + 2][1])
                if b + 3 < S:
                    loads[b + 3] = emit_load(b + 3)

    nc.compile()
    return nc


def _prep_core_inputs(x, convs, W_att, b_att):
    """Host-side shard/layout prep. Returns list of 8 per-core input dicts."""
    f32 = np.float32
    # padded input, cin split into 2 chunks of 128
    xpad = np.zeros((B, CHUNKS, 128, HP, WP), dtype=f32)
    xpad[:, :, :, 1:H + 1, 1:W + 1] = np.ascontiguousarray(x, dtype=f32).reshape(
        B, CHUNKS, 128, H, W)
    xpad = xpad.reshape(B, CHUNKS, 128, PHW)

    # convsT[half][e, c, cin, s*128 + m] = convs[e, half*128+m, c*128+cin, kh, kw]
    cv = np.ascontiguousarray(convs, dtype=f32).reshape(E, 2, MHALF, CHUNKS, 128, NSH)
    convsT_halves = [
        np.ascontiguousarray(cv[:, h].transpose(0, 2, 3, 4, 1).reshape(
            E, CHUNKS, 128, NSH * 128))
        for h in range(2)
    ]

    watt = np.ascontiguousarray(
        (np.asarray(W_att, dtype=f32).T / f32(H * W)).reshape(CHUNKS, 128, E))
    battb = np.ascontiguousarray(
        np.broadcast_to(np.asarray(b_att, dtype=f32), (128, E)))

    in_maps = []
    for k in range(NCORES):
        pair, half = k // 2, k % 2
        sl = slice(pair * SAMPLES_PER_CORE, (pair + 1) * SAMPLES_PER_CORE)
        in_maps.append({
            "xpad": np.ascontiguousarray(xpad[sl]),
            "convsT": convsT_halves[half],
            "watt": watt,
            "battb": battb,
        })
    return in_maps


def _assemble_output(results):
    out = np.empty((B, COUT, H, W), dtype=np.float32)
    for k in range(NCORES):
        pair, half = k // 2, k % 2
        sl = slice(pair * SAMPLES_PER_CORE, (pair + 1) * SAMPLES_PER_CORE)
        out[sl, half * MHALF:(half + 1) * MHALF] = results[k]["out"]
    return out


def kernel(x, convs, W_att, b_att):
    from concourse.bass_utils import run_bass_kernel_spmd

    if "nc" not in _cached:
        _cached["nc"] = _build_program()
    in_maps = _prep_core_inputs(x, convs, W_att, b_att)
    res = run_bass_kernel_spmd(_cached["nc"], in_maps, core_ids=list(range(NCORES)))
    return _assemble_output(res.results)


# revision 23
# speedup vs baseline: 1.3132x; 1.0159x over previous
"""CondConv (per-sample routed 3x3 conv) on 8 Trainium2 NeuronCores.

Reference computation (all fp32):
    gap     = mean(x, axis=(2,3))                    [B, CIN]
    routing = sigmoid(gap @ W_att.T + b_att)         [B, E]
    ker     = einsum('be,eoihw->boihw', routing, convs)
    out[b]  = conv2d(x[b], ker[b], stride 1, pad 1)  [B, COUT, 56, 56]

Sharding (B=32, COUT=256 across 8 cores): 4 core-pairs; pair p owns
samples 8p..8p+7 (batch data-parallel), and within a pair each core
computes one half of COUT (128 channels). Halving COUT per core halves
the resident expert bank so the whole pipeline stays fp32 in SBUF.

Per-core program (SPMD — same program, different data):
  - expert bank convsT [8e][2c][128cin, 9*128] resident in SBUF
  - per sample: DMA padded x -> GAP + fp32r rounding via one ScalarE
    pass -> routing on DVE/GPSIMD/ScalarE (no TensorE, so the PE queue
    is pure conv) -> VectorE mixes the per-sample kernel with fused
    scalar_tensor_tensor -> conv as 2c*9shift*7tile accumulating fp32r
    matmuls (N=448, full PE rate) -> VectorE drains PSUM -> DMA out.
  - emission is software-pipelined so mix(b+2) overlaps conv(b..b+1)
    and input DMAs run 3 samples ahead.
"""

import numpy as np

B, CIN, H, W = 32, 256, 56, 56
COUT, KK, E = 256, 3, 8
HP, WP = H + 2, W + 2          # zero-padded input plane
PHW = HP * WP                  # 3364
NSH = KK * KK                  # 9 shifts
CHUNKS = 2                     # CIN = 2 * 128
MHALF = COUT // 2              # couts per core
ROWS_PER_TILE = 8              # output rows per matmul tile
NTILES = H // ROWS_PER_TILE    # 7
NFREE = ROWS_PER_TILE * W      # 448
NCORES = 8
SAMPLES_PER_CORE = B // (NCORES // 2)  # 8

_cached = {}


def _build_program():
    import concourse.bacc as bacc
    import concourse.bass_isa as bass_isa
    import concourse.mybir as mybir
    from concourse.tile import TileContext

    f32 = mybir.dt.float32
    f32r = mybir.dt.float32r
    Alu = mybir.AluOpType
    Act = mybir.ActivationFunctionType

    nc = bacc.Bacc(None, target_bir_lowering=False)

    xpad_d = nc.declare_dram_parameter(
        "xpad", [SAMPLES_PER_CORE, CHUNKS, 128, PHW], f32r, isOutput=False)
    convsT_d = nc.declare_dram_parameter(
        "convsT", [E, CHUNKS, 128, NSH * 128], f32, isOutput=False)
    watt_d = nc.declare_dram_parameter("watt", [CHUNKS, 128, E], f32, isOutput=False)
    battb_d = nc.declare_dram_parameter("battb", [128, E], f32, isOutput=False)
    out_d = nc.declare_dram_parameter(
        "out", [SAMPLES_PER_CORE, MHALF, H, W], f32, isOutput=True)

    with TileContext(nc) as tc:
        with (
            tc.tile_pool(name="resident", bufs=1) as res_pool,
            tc.tile_pool(name="xp", bufs=3) as xp_pool,
            tc.tile_pool(name="kt", bufs=3) as kt_pool,
            tc.tile_pool(name="small", bufs=3) as small_pool,
            tc.tile_pool(name="outsb", bufs=4) as out_pool,
            tc.tile_pool(name="cpsum", bufs=1, space="PSUM") as cps_pool,
        ):
            # ---- small resident tiles -------------------------------------
            watt_sb = []
            for c in range(CHUNKS):
                t = res_pool.tile([128, E], f32, name=f"watt{c}", tag=f"watt{c}")
                nc.sync.dma_start(out=t[:], in_=watt_d[c])
                watt_sb.append(t)
            battb_sb = res_pool.tile([128, E], f32, name="battb", tag="battb")
            nc.sync.dma_start(out=battb_sb[:], in_=battb_d[:])
            # broadcast routing weights: scal[:, 8*b+e] = r_be on every partition
            scal_sb = res_pool.tile([128, SAMPLES_PER_CORE * E], f32,
                                    name="scal", tag="scal")

            convsT_sb = [[None] * CHUNKS for _ in range(E)]

            def emit_load(b):
                """DMA padded input + GAP/rounding pass for sample b.

                ScalarE in-place Copy rounds fp32 -> fp32r for the conv
                matmuls (walrus requires fp32r matmul inputs to come from a
                rounding producer) and its accum_out yields the GAP row sums
                in the same pass.
                """
                xp = []
                gq = []
                quarter = PHW // 4  # 841
                for c in range(CHUNKS):
                    t = xp_pool.tile([128, PHW], f32r, name=f"xp{c}", tag=f"xp{c}")
                    # split across 4 DMA queues, and round+accumulate each
                    # half as it lands so GAP overlaps the transfer
                    for j in range(4):
                        sl = slice(j * quarter, (j + 1) * quarter)
                        nc.sync.dma_start(out=t[:, sl], in_=xpad_d[b, c, :, sl])
                    for h in range(2):
                        sl = slice(h * 2 * quarter, (h + 1) * 2 * quarter)
                        g = small_pool.tile([128, 1], f32, name=f"gh{c}_{h}",
                                            tag=f"gh{c}_{h}")
                        nc.scalar.activation(out=t[:, sl], in_=t[:, sl],
                                             func=Act.Copy, accum_out=g[:])
                        gq.append(g)
                    xp.append(t)
                return xp, gq

            def emit_route_mix(b, gs):
                """Routing (DVE/GPSIMD/ScalarE only) + kernel mix for sample b.

                logits[e] = sum_cin gap[cin] * W_att[e,cin] / 3136 + b_att[e]
                (the 1/3136 is folded into watt host-side). Per-partition
                products on DVE, cross-partition sum on GPSIMD, sigmoid on
                ScalarE -- the TensorE queue stays pure conv.
                """
                gsum = []
                for c in range(CHUNKS):
                    g = small_pool.tile([128, 1], f32, name=f"gs{c}", tag=f"gs{c}")
                    nc.vector.tensor_add(out=g[:], in0=gs[2 * c][:],
                                         in1=gs[2 * c + 1][:])
                    gsum.append(g)
                t0 = small_pool.tile([128, E], f32, name="t0", tag="t0")
                nc.vector.tensor_scalar_mul(out=t0[:], in0=watt_sb[0][:],
                                            scalar1=gsum[0][:, 0:1])
                t1 = small_pool.tile([128, E], f32, name="t1", tag="t1")
                nc.vector.scalar_tensor_tensor(
                    out=t1[:], in0=watt_sb[1][:], scalar=gsum[1][:, 0:1],
                    in1=t0[:], op0=Alu.mult, op1=Alu.add)
                red = small_pool.tile([128, E], f32, name="red", tag="red")
                nc.gpsimd.partition_all_reduce(red[:], t1[:], channels=128,
                                               reduce_op=bass_isa.ReduceOp.add)
                red2 = small_pool.tile([128, E], f32, name="red2", tag="red2")
                nc.vector.tensor_add(out=red2[:], in0=red[:], in1=battb_sb[:])
                nc.scalar.activation(out=scal_sb[:, b * E:(b + 1) * E],
                                     in_=red2[:], func=Act.Sigmoid)

                # mix on VectorE:
                # kerT[c][cin, s*128+m] = sum_e r_be * convsT[e][c][cin, s*128+m]
                kt = []
                for c in range(CHUNKS):
                    k = kt_pool.tile([128, NSH * 128], f32r, name=f"kt{c}", tag=f"kt{c}")
                    nc.vector.tensor_scalar_mul(
                        out=k[:], in0=convsT_sb[0][c][:],
                        scalar1=scal_sb[:, b * E:b * E + 1])
                    for e in range(1, E):
                        nc.vector.scalar_tensor_tensor(
                            out=k[:], in0=convsT_sb[e][c][:],
                            scalar=scal_sb[:, b * E + e:b * E + e + 1],
                            in1=k[:], op0=Alu.mult, op1=Alu.add)
                    kt.append(k)
                return kt

            def emit_conv(b, xp, kt):
                """Conv for sample b: accumulate 2c*9shift into 7 PSUM tiles,
                then drain on VectorE and store."""
                cps = [cps_pool.tile([128, NFREE], f32, name=f"cps{n}",
                                     tag=f"cps{n}", bufs=2 if n == 0 else 1)
                       for n in range(NTILES)]
                for c in range(CHUNKS):
                    x3 = xp[c].rearrange("p (r q) -> p r q", q=WP)
                    for s in range(NSH):
                        dh, dw = s // KK, s % KK
                        lhsT = kt[c][:, s * 128:(s + 1) * 128]
                        first = (c == 0 and s == 0)
                        last = (c == CHUNKS - 1 and s == NSH - 1)
                        for n in range(NTILES):
                            rhs = x3[:, n * ROWS_PER_TILE + dh:
                                     n * ROWS_PER_TILE + dh + ROWS_PER_TILE,
                                     dw:dw + W]
                            nc.tensor.matmul(cps[n][:], lhsT, rhs,
                                             start=first, stop=last)
                # drains alternate between VectorE and ScalarE so the 7 PSUM
                # banks free up at ~2x one engine's copy rate right as the
                # next sample's matmuls want them
                for n in range(NTILES):
                    o = out_pool.tile([128, NFREE], f32, name="osb", tag="osb")
                    eng = nc.vector if n % 2 == 0 else nc.scalar
                    if eng is nc.vector:
                        eng.tensor_copy(out=o[:], in_=cps[n][:])
                    else:
                        eng.activation(out=o[:], in_=cps[n][:], func=Act.Copy)
                    nc.sync.dma_start(
                        out=out_d[b, :, n * ROWS_PER_TILE:(n + 1) * ROWS_PER_TILE, :],
                        in_=o[:])

            # ---- software-pipelined emission ------------------------------
            # All loads ride the same HWDGE rings as the conv bank, so queue
            # FIFO order enforces: xp(0), bank chunk0, bank chunk1, xp(1),
            # xp(2), then steady-state loads 3 samples ahead. routing+mix(b+2)
            # is emitted after conv(b) so the mix overlaps conv(b) on VectorE,
            # with drains at the queue head for prompt PSUM recycling.
            S = SAMPLES_PER_CORE
            loads = {0: emit_load(0)}
            for c in range(CHUNKS):
                for e in range(E):
                    t = res_pool.tile([128, NSH * 128], f32,
                                      name=f"cv_{e}_{c}", tag=f"cv_{e}_{c}")
                    nc.sync.dma_start(out=t[:], in_=convsT_d[e, c])
                    convsT_sb[e][c] = t
            kts = {0: emit_route_mix(0, loads[0][1])}
            loads[1] = emit_load(1)
            kts[1] = emit_route_mix(1, loads[1][1])
            loads[2] = emit_load(2)
            for b in range(S):
                emit_conv(b, loads.pop(b)[0], kts.pop(b))
                if b + 2 < S:
                    kts[b + 2] = emit_route_mix(b + 2, loads[b + 2][1])
                if b + 3 < S:
                    loads[b + 3] = emit_load(b + 3)

    nc.compile()
    return nc


def _prep_core_inputs(x, convs, W_att, b_att):
    """Host-side shard/layout prep. Returns list of 8 per-core input dicts."""
    f32 = np.float32
    # padded input, cin split into 2 chunks of 128
    xpad = np.zeros((B, CHUNKS, 128, HP, WP), dtype=f32)
    xpad[:, :, :, 1:H + 1, 1:W + 1] = np.ascontiguousarray(x, dtype=f32).reshape(
        B, CHUNKS, 128, H, W)
    xpad = xpad.reshape(B, CHUNKS, 128, PHW)

    # convsT[half][e, c, cin, s*128 + m] = convs[e, half*128+m, c*128+cin, kh, kw]
    cv = np.ascontiguousarray(convs, dtype=f32).reshape(E, 2, MHALF, CHUNKS, 128, NSH)
    convsT_halves = [
        np.ascontiguousarray(cv[:, h].transpose(0, 2, 3, 4, 1).reshape(
            E, CHUNKS, 128, NSH * 128))
        for h in range(2)
    ]

    watt = np.ascontiguousarray(
        (np.asarray(W_att, dtype=f32).T / f32(H * W)).reshape(CHUNKS, 128, E))
    battb = np.ascontiguousarray(
        np.broadcast_to(np.asarray(b_att, dtype=f32), (128, E)))

    in_maps = []
    for k in range(NCORES):
        pair, half = k // 2, k % 2
        sl = slice(pair * SAMPLES_PER_CORE, (pair + 1) * SAMPLES_PER_CORE)
        in_maps.append({
            "xpad": np.ascontiguousarray(xpad[sl]),
            "convsT": convsT_halves[half],
            "watt": watt,
            "battb": battb,
        })
    return in_maps


def _assemble_output(results):
    out = np.empty((B, COUT, H, W), dtype=np.float32)
    for k in range(NCORES):
        pair, half = k // 2, k % 2
        sl = slice(pair * SAMPLES_PER_CORE, (pair + 1) * SAMPLES_PER_CORE)
        out[sl, half * MHALF:(half + 1) * MHALF] = results[k]["out"]
    return out


def kernel(x, convs, W_att, b_att):
    from concourse.bass_utils import run_bass_kernel_spmd

    if "nc" not in _cached:
        _cached["nc"] = _build_program()
    in_maps = _prep_core_inputs(x, convs, W_att, b_att)
    res = run_bass_kernel_spmd(_cached["nc"], in_maps, core_ids=list(range(NCORES)))
    return _assemble_output(res.results)


# revision 27
# speedup vs baseline: 1.3284x; 1.0116x over previous
"""CondConv (per-sample routed 3x3 conv) on 8 Trainium2 NeuronCores.

Reference computation (all fp32):
    gap     = mean(x, axis=(2,3))                    [B, CIN]
    routing = sigmoid(gap @ W_att.T + b_att)         [B, E]
    ker     = einsum('be,eoihw->boihw', routing, convs)
    out[b]  = conv2d(x[b], ker[b], stride 1, pad 1)  [B, COUT, 56, 56]

Sharding (B=32, COUT=256 across 8 cores): 4 core-pairs; pair p owns
samples 8p..8p+7 (batch data-parallel), and within a pair each core
computes one half of COUT (128 channels). Halving COUT per core halves
the resident expert bank so the whole pipeline stays fp32 in SBUF.

Per-core program (SPMD — same program, different data):
  - expert bank convsT [8e][2c][128cin, 9*128] resident in SBUF
  - per sample: DMA padded x -> GAP + fp32r rounding via one ScalarE
    pass -> routing on DVE/GPSIMD/ScalarE (no TensorE, so the PE queue
    is pure conv) -> VectorE mixes the per-sample kernel with fused
    scalar_tensor_tensor -> conv as 2c*9shift*7tile accumulating fp32r
    matmuls (N=448, full PE rate) -> VectorE drains PSUM -> DMA out.
  - emission is software-pipelined so mix(b+2) overlaps conv(b..b+1)
    and input DMAs run 3 samples ahead.
"""

import numpy as np

B, CIN, H, W = 32, 256, 56, 56
COUT, KK, E = 256, 3, 8
HP, WP = H + 2, W + 2          # zero-padded input plane
PHW = HP * WP                  # 3364
NSH = KK * KK                  # 9 shifts
CHUNKS = 2                     # CIN = 2 * 128
MHALF = COUT // 2              # couts per core
ROWS_PER_TILE = 8              # output rows per matmul tile
NTILES = H // ROWS_PER_TILE    # 7
NFREE = ROWS_PER_TILE * W      # 448
NCORES = 8
SAMPLES_PER_CORE = B // (NCORES // 2)  # 8

_cached = {}


def _build_program():
    import concourse.bacc as bacc
    import concourse.bass_isa as bass_isa
    import concourse.mybir as mybir
    from concourse.tile import TileContext

    f32 = mybir.dt.float32
    f32r = mybir.dt.float32r
    Alu = mybir.AluOpType
    Act = mybir.ActivationFunctionType

    nc = bacc.Bacc(None, target_bir_lowering=False)

    xpad_d = nc.declare_dram_parameter(
        "xpad", [SAMPLES_PER_CORE, CHUNKS, 128, PHW], f32r, isOutput=False)
    convsT_d = nc.declare_dram_parameter(
        "convsT", [E, CHUNKS, 128, NSH * 128], f32, isOutput=False)
    watt_d = nc.declare_dram_parameter("watt", [CHUNKS, 128, E], f32, isOutput=False)
    battb_d = nc.declare_dram_parameter("battb", [128, E], f32, isOutput=False)
    out_d = nc.declare_dram_parameter(
        "out", [SAMPLES_PER_CORE, MHALF, H, W], f32, isOutput=True)

    with TileContext(nc) as tc:
        with (
            tc.tile_pool(name="resident", bufs=1) as res_pool,
            tc.tile_pool(name="xp", bufs=3) as xp_pool,
            tc.tile_pool(name="kt", bufs=3) as kt_pool,
            tc.tile_pool(name="small", bufs=3) as small_pool,
            tc.tile_pool(name="outsb", bufs=4) as out_pool,
            tc.tile_pool(name="cpsum", bufs=1, space="PSUM") as cps_pool,
        ):
            # ---- small resident tiles -------------------------------------
            watt_sb = []
            for c in range(CHUNKS):
                t = res_pool.tile([128, E], f32, name=f"watt{c}", tag=f"watt{c}")
                nc.sync.dma_start(out=t[:], in_=watt_d[c])
                watt_sb.append(t)
            battb_sb = res_pool.tile([128, E], f32, name="battb", tag="battb")
            nc.sync.dma_start(out=battb_sb[:], in_=battb_d[:])
            # broadcast routing weights: scal[:, 8*b+e] = r_be on every partition
            scal_sb = res_pool.tile([128, SAMPLES_PER_CORE * E], f32,
                                    name="scal", tag="scal")

            convsT_sb = [[None] * CHUNKS for _ in range(E)]

            def emit_load_dma(b):
                """DMA padded input for sample b, split across 4 queues."""
                xp = []
                quarter = PHW // 4  # 841
                for c in range(CHUNKS):
                    t = xp_pool.tile([128, PHW], f32r, name=f"xp{c}", tag=f"xp{c}")
                    for j in range(4):
                        sl = slice(j * quarter, (j + 1) * quarter)
                        nc.sync.dma_start(out=t[:, sl], in_=xpad_d[b, c, :, sl])
                    xp.append(t)
                return xp

            def emit_load_gap(xp):
                """GAP/rounding pass: ScalarE in-place Copy rounds fp32 ->
                fp32r for the conv matmuls (walrus requires fp32r matmul
                inputs to come from a rounding producer) and its accum_out
                yields the GAP row sums. Split in halves so it overlaps the
                input DMA."""
                gq = []
                half = PHW // 2
                for c in range(CHUNKS):
                    for h in range(2):
                        sl = slice(h * half, (h + 1) * half)
                        g = small_pool.tile([128, 1], f32, name=f"gh{c}_{h}",
                                            tag=f"gh{c}_{h}")
                        nc.scalar.activation(out=xp[c][:, sl], in_=xp[c][:, sl],
                                             func=Act.Copy, accum_out=g[:])
                        gq.append(g)
                return gq

            def emit_load(b):
                xp = emit_load_dma(b)
                return xp, emit_load_gap(xp)

            def emit_routing(b, gs):
                """Routing for sample b on DVE/GPSIMD/ScalarE only.

                logits[e] = sum_cin gap[cin] * W_att[e,cin] / 3136 + b_att[e]
                (the 1/3136 is folded into watt host-side). Per-partition
                products on DVE, cross-partition sum on GPSIMD, sigmoid on
                ScalarE -- the TensorE queue stays pure conv.
                """
                gsum = []
                for c in range(CHUNKS):
                    g = small_pool.tile([128, 1], f32, name=f"gs{c}", tag=f"gs{c}")
                    nc.vector.tensor_add(out=g[:], in0=gs[2 * c][:],
                                         in1=gs[2 * c + 1][:])
                    gsum.append(g)
                t0 = small_pool.tile([128, E], f32, name="t0", tag="t0")
                nc.vector.tensor_scalar_mul(out=t0[:], in0=watt_sb[0][:],
                                            scalar1=gsum[0][:, 0:1])
                t1 = small_pool.tile([128, E], f32, name="t1", tag="t1")
                nc.vector.scalar_tensor_tensor(
                    out=t1[:], in0=watt_sb[1][:], scalar=gsum[1][:, 0:1],
                    in1=t0[:], op0=Alu.mult, op1=Alu.add)
                red = small_pool.tile([128, E], f32, name="red", tag="red")
                nc.gpsimd.partition_all_reduce(red[:], t1[:], channels=128,
                                               reduce_op=bass_isa.ReduceOp.add)
                red2 = small_pool.tile([128, E], f32, name="red2", tag="red2")
                nc.vector.tensor_add(out=red2[:], in0=red[:], in1=battb_sb[:])
                nc.scalar.activation(out=scal_sb[:, b * E:(b + 1) * E],
                                     in_=red2[:], func=Act.Sigmoid)

            def emit_mix_chunk(b, c):
                """Mix chunk c of sample b's kernel on VectorE:
                kerT[c][cin, s*128+m] = sum_e r_be * convsT[e][c][cin, s*128+m]
                """
                k = kt_pool.tile([128, NSH * 128], f32r, name=f"kt{c}", tag=f"kt{c}")
                nc.vector.tensor_scalar_mul(
                    out=k[:], in0=convsT_sb[0][c][:],
                    scalar1=scal_sb[:, b * E:b * E + 1])
                for e in range(1, E):
                    nc.vector.scalar_tensor_tensor(
                        out=k[:], in0=convsT_sb[e][c][:],
                        scalar=scal_sb[:, b * E + e:b * E + e + 1],
                        in1=k[:], op0=Alu.mult, op1=Alu.add)
                return k

            def emit_route_mix(b, gs):
                emit_routing(b, gs)
                return [emit_mix_chunk(b, c) for c in range(CHUNKS)]

            def emit_conv(b, xp, kt):
                """Conv for sample b: accumulate 2c*9shift into 7 PSUM tiles,
                then drain on VectorE and store."""
                cps = [cps_pool.tile([128, NFREE], f32, name=f"cps{n}",
                                     tag=f"cps{n}", bufs=2 if n == 0 else 1)
                       for n in range(NTILES)]
                for c in range(CHUNKS):
                    x3 = xp[c].rearrange("p (r q) -> p r q", q=WP)
                    for s in range(NSH):
                        dh, dw = s // KK, s % KK
                        lhsT = kt[c][:, s * 128:(s + 1) * 128]
                        first = (c == 0 and s == 0)
                        last = (c == CHUNKS - 1 and s == NSH - 1)
                        for n in range(NTILES):
                            rhs = x3[:, n * ROWS_PER_TILE + dh:
                                     n * ROWS_PER_TILE + dh + ROWS_PER_TILE,
                                     dw:dw + W]
                            nc.tensor.matmul(cps[n][:], lhsT, rhs,
                                             start=first, stop=last)
                # drains alternate between VectorE and ScalarE so the 7 PSUM
                # banks free up at ~2x one engine's copy rate right as the
                # next sample's matmuls want them
                for n in range(NTILES):
                    o = out_pool.tile([128, NFREE], f32, name="osb", tag="osb")
                    eng = nc.vector if n % 2 == 0 else nc.scalar
                    if eng is nc.vector:
                        eng.tensor_copy(out=o[:], in_=cps[n][:])
                    else:
                        eng.activation(out=o[:], in_=cps[n][:], func=Act.Copy)
                    nc.sync.dma_start(
                        out=out_d[b, :, n * ROWS_PER_TILE:(n + 1) * ROWS_PER_TILE, :],
                        in_=o[:])

            # ---- software-pipelined emission ------------------------------
            # All loads ride the same HWDGE rings as the conv bank, so queue
            # FIFO order enforces: xp(0), bank chunk0, xp(1), bank chunk1,
            # xp(2), then steady-state loads 3 samples ahead. routing+mix of
            # sample b+2 is emitted after conv(b) so the mix overlaps conv(b)
            # on VectorE, with drains at the queue head for prompt PSUM
            # recycling. The prologue hand-orders sample 0/1 mix chunks
            # around the bank-chunk arrivals.
            S = SAMPLES_PER_CORE

            def emit_bank_chunk(c):
                for e in range(E):
                    t = res_pool.tile([128, NSH * 128], f32,
                                      name=f"cv_{e}_{c}", tag=f"cv_{e}_{c}")
                    nc.sync.dma_start(out=t[:], in_=convsT_d[e, c])
                    convsT_sb[e][c] = t

            loads = {0: emit_load(0)}
            emit_bank_chunk(0)
            emit_routing(0, loads[0][1])
            kt0c0 = emit_mix_chunk(0, 0)
            xp1 = emit_load_dma(1)
            emit_bank_chunk(1)
            kt0c1 = emit_mix_chunk(0, 1)
            kts = {0: [kt0c0, kt0c1]}
            loads[1] = (xp1, emit_load_gap(xp1))
            emit_routing(1, loads[1][1])
            kt1c0 = emit_mix_chunk(1, 0)
            loads[2] = emit_load(2)
            emit_conv(0, loads.pop(0)[0], kts.pop(0))
            kts[1] = [kt1c0, emit_mix_chunk(1, 1)]
            for b in range(1, S):
                if b + 1 < S and b + 1 not in kts:
                    kts[b + 1] = emit_route_mix(b + 1, loads[b + 1][1])
                if b + 2 < S:
                    loads[b + 2] = emit_load(b + 2)
                emit_conv(b, loads.pop(b)[0], kts.pop(b))

    nc.compile()
    return nc


def _prep_core_inputs(x, convs, W_att, b_att):
    """Host-side shard/layout prep. Returns list of 8 per-core input dicts."""
    f32 = np.float32
    # padded input, cin split into 2 chunks of 128
    xpad = np.zeros((B, CHUNKS, 128, HP, WP), dtype=f32)
    xpad[:, :, :, 1:H + 1, 1:W + 1] = np.ascontiguousarray(x, dtype=f32).reshape(
        B, CHUNKS, 128, H, W)
    xpad = xpad.reshape(B, CHUNKS, 128, PHW)

    # convsT[half][e, c, cin, s*128 + m] = convs[e, half*128+m, c*128+cin, kh, kw]
    cv = np.ascontiguousarray(convs, dtype=f32).reshape(E, 2, MHALF, CHUNKS, 128, NSH)
    convsT_halves = [
        np.ascontiguousarray(cv[:, h].transpose(0, 2, 3, 4, 1).reshape(
            E, CHUNKS, 128, NSH * 128))
        for h in range(2)
    ]

    watt = np.ascontiguousarray(
        (np.asarray(W_att, dtype=f32).T / f32(H * W)).reshape(CHUNKS, 128, E))
    battb = np.ascontiguousarray(
        np.broadcast_to(np.asarray(b_att, dtype=f32), (128, E)))

    in_maps = []
    for k in range(NCORES):
        pair, half = k // 2, k % 2
        sl = slice(pair * SAMPLES_PER_CORE, (pair + 1) * SAMPLES_PER_CORE)
        in_maps.append({
            "xpad": np.ascontiguousarray(xpad[sl]),
            "convsT": convsT_halves[half],
            "watt": watt,
            "battb": battb,
        })
    return in_maps


def _assemble_output(results):
    out = np.empty((B, COUT, H, W), dtype=np.float32)
    for k in range(NCORES):
        pair, half = k // 2, k % 2
        sl = slice(pair * SAMPLES_PER_CORE, (pair + 1) * SAMPLES_PER_CORE)
        out[sl, half * MHALF:(half + 1) * MHALF] = results[k]["out"]
    return out


def kernel(x, convs, W_att, b_att):
    from concourse.bass_utils import run_bass_kernel_spmd

    if "nc" not in _cached:
        _cached["nc"] = _build_program()
    in_maps = _prep_core_inputs(x, convs, W_att, b_att)
    res = run_bass_kernel_spmd(_cached["nc"], in_maps, core_ids=list(range(NCORES)))
    return _assemble_output(res.results)
